# revision 18
# baseline (speedup 1.0000x reference)
"""Trainium2 Bass kernel for Llama-style GQA attention (B=1, S=2048, D=4096,
32 Q heads / 8 KV heads, head_dim 128, RoPE, additive mask, causal-aware).

Sharding: 8-way tensor-parallel over heads. Core c computes Q heads 4c..4c+3
and KV head c end-to-end (projections + RoPE + attention + its rows of wo),
producing a partial [S, D] output; the host sums the 8 partials (the
all-reduce of the row-parallel wo).

Device layout strategy (all fp32):
  - Host feeds xT = x.T so Q/K projections produce qT/kT ([head_dim, s]) and
    the V projection produces vT, with zero on-device transposes of x.
  - RoPE's even/odd interleave is folded into a column permutation of wq/wk
    (scores are invariant under a shared permutation of q and k), making RoPE
    pure partition-aligned elementwise math: rows 0:64 = "real", 64:128 =
    "imag" components, cos/sin fed pre-transposed.
  - Scores are computed transposed: ST[sk, sq] = K @ Q^T. Softmax reduction
    over sk (partitions) is a ones-vector matmul; probabilities feed the PV
    matmul directly as rhs (ctxT = V^T @ expST) with no transposition.
  - ctxT is exactly the lhsT the wo matmul needs. 1/sqrt(head_dim) is folded
    into wq on the host. Softmax uses exp without max subtraction (scores are
    O(1) for this problem's input distribution) and multiplicative exp(mask)
    block patterns, deduplicated and usually resolved to skip/plain.
"""

import math
import numpy as np


def _rne11(x):
    """Round fp32 to the float32r grid (RNE at 11 mantissa bits)."""
    b = x.view(np.uint32).astype(np.uint64)
    bias = ((b >> 12) & 1) + 0x7FF
    return ((b + bias) >> 12 << 12).astype(np.uint32).view(np.float32)

P = 128          # SBUF partitions / head_dim / tile edge
S = 2048         # sequence length
D = 4096         # model dim
HD = 128         # head dim
N_HEADS = 32
N_KV = 8
N_CORES = 8
NH_LOC = N_HEADS // N_CORES   # 4 local Q heads
SG = 512         # score/free-dim group width (one PSUM bank of fp32)
NG = S // SG     # 4 q-position groups
KT = D // P      # 32 contraction tiles for projections
NSK = S // P     # 16 key tiles

_CACHE = {}


def _classify_mask(mask):
    """Classify each [P, SG] block of mask.T into skip / plain / masked.

    Returns (sk_lists, patterns):
      sk_lists[G] = list of (m, pat_idx_or_None) key-tiles to compute for
                    query group G, and patterns = [P, SG] multiplicative
                    exp(mask) blocks (deduped).
    """
    mt = np.ascontiguousarray(mask.T.astype(np.float32))
    patterns = []
    pat_idx = {}
    sk_lists = []
    for G in range(NG):
        lst = []
        for m in range(NSK):
            blk = mt[m * P:(m + 1) * P, G * SG:(G + 1) * SG]
            if np.all(np.isneginf(blk)):
                continue
            if np.all(blk == 0.0):
                lst.append((m, None))
                continue
            with np.errstate(over="ignore"):
                pat = np.exp(blk).astype(np.float32)
            key = pat.tobytes()
            if key not in pat_idx:
                pat_idx[key] = len(patterns)
                patterns.append(pat)
            lst.append((m, pat_idx[key]))
        # masked (usually diagonal) tiles first: their extra mask-multiply
        # latency hides behind the plain tiles that follow
        lst.sort(key=lambda t: t[1] is None)
        sk_lists.append(lst)
    return sk_lists, patterns


def _build_program(sk_lists, n_pat):
    import concourse.tile as tile
    from concourse import bacc, mybir
    from concourse.masks import make_identity
    from contextlib import ExitStack

    f32 = mybir.dt.float32
    f32r = mybir.dt.float32r
    Exp = mybir.ActivationFunctionType.Exp

    nc = bacc.Bacc()
    XWB = SG + NH_LOC * HD        # one fused x|wq block: 1024 cols
    xw_d = nc.dram_tensor("xw", [P, NG * KT * XWB], f32r, kind="ExternalInput")
    wk_d = nc.dram_tensor("wk", [P, KT * HD], f32r, kind="ExternalInput")
    wv_d = nc.dram_tensor("wv", [P, KT * HD], f32r, kind="ExternalInput")
    wo_d = nc.dram_tensor("wo", [P, (D // SG) * NH_LOC * SG], f32r,
                          kind="ExternalInput")
    cs_d = nc.dram_tensor("cs", [P, S], f32, kind="ExternalInput")
    mb_d = None
    if n_pat:
        mb_d = nc.dram_tensor("mb", [n_pat, P, SG], f32r, kind="ExternalInput")
    out_d = nc.dram_tensor("out", [S, D], f32, kind="ExternalOutput")

    with ExitStack() as ctx:
        tc = ctx.enter_context(tile.TileContext(nc))
        consts = ctx.enter_context(tc.tile_pool(name="consts", bufs=1))
        kv = ctx.enter_context(tc.tile_pool(name="kv", bufs=1))
        xp = ctx.enter_context(tc.tile_pool(name="xp", bufs=4))
        qp = ctx.enter_context(tc.tile_pool(name="qp", bufs=1))
        rp = ctx.enter_context(tc.tile_pool(name="rp", bufs=4))
        ep = ctx.enter_context(tc.tile_pool(name="ep", bufs=4))
        sp = ctx.enter_context(tc.tile_pool(name="sp", bufs=4))
        cp = ctx.enter_context(tc.tile_pool(name="cp", bufs=8))
        ps = ctx.enter_context(tc.tile_pool(name="ps", bufs=8, space="PSUM"))

        # resident weights / constants (wq is streamed per-use; too big).
        # Consts ride the ACT DMA ring so the x|wq stream owns the SP ring.
        wk_sb = consts.tile([P, KT * HD], f32r)
        wv_sb = consts.tile([P, KT * HD], f32r)
        qtr = KT * HD // 4
        for i in range(4):
            nc.scalar.dma_start(wk_sb[:, i * qtr:(i + 1) * qtr],
                                wk_d[:, i * qtr:(i + 1) * qtr])
            nc.scalar.dma_start(wv_sb[:, i * qtr:(i + 1) * qtr],
                                wv_d[:, i * qtr:(i + 1) * qtr])
        cs_sb = consts.tile([P, S], f32)
        nc.scalar.dma_start(cs_sb[:], cs_d[:, :])
        mb_sb = None
        if n_pat:
            mb_sb = consts.tile([P, n_pat * SG], f32r)
            for i in range(n_pat):
                nc.scalar.dma_start(mb_sb[:, i * SG:(i + 1) * SG], mb_d[i])
        ones_f = consts.tile([P, 1], f32)
        nc.vector.memset(ones_f[:], 1.0)
        ones_col = consts.tile([P, 1], f32r)
        nc.vector.tensor_copy(ones_col[:], ones_f[:])
        ones_row = consts.tile([1, P], f32)
        nc.vector.memset(ones_row[:], 1.0)
        ident = consts.tile([P, P], f32)
        make_identity(nc, ident[:])

        # full-sequence KV + context accumulators
        kT_sb = kv.tile([P, S], f32r)                # [head_dim', s]
        v_sb = kv.tile([P, S], f32r)                # [s%P, (s//P)*HD + hd]
        ctx_sb = kv.tile([P, NH_LOC * S], f32r)       # [hd, h*S + sq]

        # pending per-head softmax finalization, emitted later so the PE
        # queue never stalls on the reciprocal chain (in-order engine)
        def finalize(fin):
            cacc, sacc, h, G0 = fin
            inv = sp.tile([1, SG], f32, tag="inv", bufs=2)
            nc.vector.reciprocal(inv[:], sacc[:])
            bc = ps.tile([P, SG], f32, tag="bank", bufs=8, name="bc")
            nc.tensor.matmul(bc[:], ones_row[:], inv[:], start=True, stop=True)
            bcs = sp.tile([P, SG], f32, tag="bcs", bufs=2)
            nc.vector.tensor_copy(bcs[:], bc[:])
            nc.vector.tensor_mul(
                ctx_sb[:, h * S + G0 * SG:h * S + (G0 + 1) * SG],
                cacc[:], bcs[:])

        pending = None
        for G in range(NG):
            gsl = slice(G * SG, (G + 1) * SG)
            # ---------------- phase A: projections for s-slice G -----------
            pq = [ps.tile([P, SG], f32, tag="bank", bufs=8, name=f"pq{_l}")
                  for _l in range(NH_LOC)]
            pk = ps.tile([P, SG], f32, tag="bank", bufs=8, name="pk")
            pv = ps.tile([P, SG], f32, tag="bank", bufs=8, name="pv")
            for k2 in range(KT // 2):
                xw = xp.tile([P, 2 * XWB], f32r, tag="xw", bufs=5, name="xw")
                blk = (G * KT + 2 * k2) * XWB
                ring = nc.sync if k2 % 2 == 0 else nc.scalar
                ring.dma_start(xw[:], xw_d[:, blk:blk + 2 * XWB])
                for k in (2 * k2, 2 * k2 + 1):
                    off = (k - 2 * k2) * XWB
                    xt = xw[:, off:off + SG]
                    st_k, sp_k = (k == 0), (k == KT - 1)
                    for l in range(NH_LOC):
                        nc.tensor.matmul(
                            pq[l][:],
                            xw[:, off + SG + l * HD:off + SG + (l + 1) * HD],
                            xt, start=st_k, stop=sp_k)
                    nc.tensor.matmul(pk[:], wk_sb[:, k * HD:(k + 1) * HD], xt,
                                     start=st_k, stop=sp_k)
                    nc.tensor.matmul(pv[:], wv_sb[:, k * HD:(k + 1) * HD], xt,
                                     start=st_k, stop=sp_k)

            if pending is not None:     # head 3 of the previous group
                finalize(pending)
                pending = None

            # RoPE (rows 0:64 real, 64:128 imag), PSUM -> SBUF.
            # Order q0 first then k: B(G, h=0) only needs q0 (+ kT for the
            # diagonal tiles, needed first only at G=0).
            qts = [None] * NH_LOC
            cos = cs_sb[0:64, gsl]
            sin = cs_sb[64:128, gsl]
            for l in (0, NH_LOC, 1, 2, 3):
                src = pq[l] if l < NH_LOC else pk
                if l < NH_LOC:
                    dst = qp.tile([P, SG], f32r, tag="qT", bufs=6, name="qT")
                    qts[l] = dst
                    dr, di = dst[0:64, :], dst[64:128, :]
                else:
                    dr, di = kT_sb[0:64, gsl], kT_sb[64:128, gsl]
                ta = rp.tile([64, SG], f32, tag="ropeA", bufs=2)
                tb = rp.tile([64, SG], f32, tag="ropeB", bufs=2)
                tcc = rp.tile([64, SG], f32, tag="ropeC", bufs=2)
                td = rp.tile([64, SG], f32, tag="ropeD", bufs=2)
                nc.vector.tensor_mul(ta[:], src[0:64, :], cos)
                nc.vector.tensor_mul(tcc[:], src[0:64, :], sin)
                nc.vector.tensor_mul(tb[:], src[64:128, :], sin)
                nc.vector.tensor_mul(td[:], src[64:128, :], cos)
                nc.vector.tensor_sub(dr, ta[:], tb[:])
                nc.vector.tensor_add(di, tcc[:], td[:])

            # vT -> v (PE transpose via identity)
            vt = sp.tile([P, SG], f32, tag="vtmp", bufs=2)
            nc.scalar.copy(vt[:], pv[:])
            for j in range(SG // P):
                ptr = ps.tile([P, P], f32, tag="bank", bufs=8, name="ptr")
                nc.tensor.transpose(ptr[:], vt[:, j * P:(j + 1) * P], ident[:])
                vdst = v_sb[:, (G * 4 + j) * HD:(G * 4 + j + 1) * HD]
                if j % 2:
                    nc.scalar.copy(vdst, ptr[:])
                else:
                    nc.vector.tensor_copy(vdst, ptr[:])

            # ---------------- phase B: attention for q-group G -------------
            for h in range(NH_LOC):
                cacc = ps.tile([P, SG], f32, tag="bank", bufs=8, name="cacc")
                sacc = ps.tile([1, SG], f32, tag="bank", bufs=8, name="sacc")
                n_sk = len(sk_lists[G])
                for idx, (m, pat) in enumerate(sk_lists[G]):
                    stp = ps.tile([P, SG], f32, tag="bank", bufs=8, name="stp")
                    nc.tensor.matmul(stp[:], kT_sb[:, m * P:(m + 1) * P],
                                     qts[h][:], start=True, stop=True)
                    ex = ep.tile([P, SG], f32r, tag="ex", bufs=3)
                    nc.scalar.activation(ex[:], stp[:], Exp)
                    if pat is not None:
                        nc.gpsimd.tensor_mul(
                            ex[:], ex[:], mb_sb[:, pat * SG:(pat + 1) * SG])
                    st_a, sp_a = (idx == 0), (idx == n_sk - 1)
                    nc.tensor.matmul(cacc[:], v_sb[:, m * HD:(m + 1) * HD],
                                     ex[:], start=st_a, stop=sp_a)
                    nc.tensor.matmul(sacc[:], ones_col[:], ex[:],
                                     start=st_a, stop=sp_a)
                if pending is not None:
                    finalize(pending)
                pending = (cacc, sacc, h, G)
        finalize(pending)

        # ---------------- phase C: out = ctx @ wo (partial) ----------------
        for n in range(D // SG):
            wt = cp.tile([P, NH_LOC * SG], f32r, tag="wo", bufs=2, name="wot")
            nc.scalar.dma_start(
                wt[:], wo_d[:, n * NH_LOC * SG:(n + 1) * NH_LOC * SG])
            for m in range(NSK):
                po = ps.tile([P, SG], f32, tag="bank", bufs=8, name="po")
                for kk in range(NH_LOC):
                    nc.tensor.matmul(po[:],
                                     ctx_sb[:, kk * S + m * P:kk * S + (m + 1) * P],
                                     wt[:, kk * SG:(kk + 1) * SG],
                                     start=(kk == 0), stop=(kk == NH_LOC - 1))
                ot = cp.tile([P, SG], f32, tag="ot", bufs=3)
                if m % 2:
                    nc.scalar.copy(ot[:], po[:])
                else:
                    nc.vector.tensor_copy(ot[:], po[:])
                nc.sync.dma_start(out_d[m * P:(m + 1) * P, n * SG:(n + 1) * SG], ot[:])

    nc.compile()
    return nc


def _host_prep(x, wq, wk, wv, wo, freqs_cos, freqs_sin):
    """Build per-core input maps (all layouts pre-tiled for contiguous DMA)."""
    x = np.ascontiguousarray(np.asarray(x, dtype=np.float32).reshape(S, D))
    wq = np.asarray(wq, dtype=np.float32)
    wk = np.asarray(wk, dtype=np.float32)
    wv = np.asarray(wv, dtype=np.float32)
    wo = np.asarray(wo, dtype=np.float32)

    perm = np.concatenate([np.arange(0, HD, 2), np.arange(1, HD, 2)])
    scale = 1.0 / math.sqrt(HD)
    wq_p = (wq.reshape(D, N_HEADS, HD)[:, :, perm] * scale).astype(np.float32)
    wk_p = wk.reshape(D, N_KV, HD)[:, :, perm]

    # xT blocks: xtb[p, G, k, c] = x[G*SG + c, k*P + p]
    xtb = _rne11(np.ascontiguousarray(
        x.T.reshape(KT, P, NG, SG).transpose(1, 2, 0, 3)))   # [P, NG, KT, SG]
    cs = np.ascontiguousarray(
        np.concatenate([np.asarray(freqs_cos, np.float32).T,
                        np.asarray(freqs_sin, np.float32).T], axis=0))

    in_maps = []
    for c in range(N_CORES):
        wq_c = wq_p[:, 4 * c:4 * c + 4, :].reshape(D, NH_LOC * HD)
        wq_l = _rne11(np.ascontiguousarray(
            wq_c.reshape(KT, P, NH_LOC * HD).transpose(1, 0, 2)))  # [P, KT, 512]
        # fused x|wq stream: block (G, k) = [ xT(G,k) 512 | wq(k) 512 ]
        xw = np.empty((P, NG, KT, SG + NH_LOC * HD), np.float32)
        xw[:, :, :, :SG] = xtb
        xw[:, :, :, SG:] = wq_l[:, None, :, :]
        xw = np.ascontiguousarray(xw.reshape(P, NG * KT * (SG + NH_LOC * HD)))
        wk_c = wk_p[:, c, :]
        wk_l = np.ascontiguousarray(
            wk_c.reshape(KT, P, HD).transpose(1, 0, 2).reshape(P, KT * HD))
        wv_c = wv.reshape(D, N_KV, HD)[:, c, :]
        wv_l = np.ascontiguousarray(
            wv_c.reshape(KT, P, HD).transpose(1, 0, 2).reshape(P, KT * HD))
        wo_c = wo[4 * c * HD:(4 * c + 4) * HD, :]       # [512, D]
        # [P, n, kk, 512]: per dim-group n, the 4 head-chunk tiles adjacent
        wo_l = np.ascontiguousarray(
            wo_c.reshape(NH_LOC, P, D // SG, SG).transpose(1, 2, 0, 3)
            .reshape(P, (D // SG) * NH_LOC * SG))
        in_maps.append({"xw": xw, "wk": _rne11(wk_l),
                        "wv": _rne11(wv_l), "wo": _rne11(wo_l), "cs": cs})
    return in_maps


def _run(x, wq, wk, wv, wo, freqs_cos, freqs_sin, mask, start_pos, trace=False):
    assert int(start_pos) == 0
    sk_lists, patterns = _classify_mask(np.asarray(mask, dtype=np.float32))
    n_pat = len(patterns)
    fp = (tuple(tuple(lst) for lst in sk_lists), n_pat)

    if fp not in _CACHE:
        _CACHE[fp] = _build_program(sk_lists, n_pat)
    nc = _CACHE[fp]

    in_maps = _host_prep(x, wq, wk, wv, wo, freqs_cos, freqs_sin)
    if n_pat:
        mb = _rne11(np.ascontiguousarray(np.stack(patterns)))
        for m in in_maps:
            m["mb"] = mb

    from concourse.bass_utils import run_bass_kernel_spmd
    res = run_bass_kernel_spmd(nc, in_maps, list(range(N_CORES)), trace=trace)
    out = np.zeros((S, D), dtype=np.float32)
    for c in range(N_CORES):
        out += res.results[c]["out"]
    return out.reshape(1, S, D), res


def kernel(x, wq, wk, wv, wo, freqs_cos, freqs_sin, mask, start_pos):
    out, _ = _run(x, wq, wk, wv, wo, freqs_cos, freqs_sin, mask, start_pos)
    return out


# revision 19
# speedup vs baseline: 1.0067x; 1.0067x over previous
"""Trainium2 Bass kernel for Llama-style GQA attention (B=1, S=2048, D=4096,
32 Q heads / 8 KV heads, head_dim 128, RoPE, additive mask, causal-aware).

Sharding: 8-way tensor-parallel over heads. Core c computes Q heads 4c..4c+3
and KV head c end-to-end (projections + RoPE + attention + its rows of wo),
producing a partial [S, D] output; the host sums the 8 partials (the
all-reduce of the row-parallel wo).

Device layout strategy (all fp32):
  - Host feeds xT = x.T so Q/K projections produce qT/kT ([head_dim, s]) and
    the V projection produces vT, with zero on-device transposes of x.
  - RoPE's even/odd interleave is folded into a column permutation of wq/wk
    (scores are invariant under a shared permutation of q and k), making RoPE
    pure partition-aligned elementwise math: rows 0:64 = "real", 64:128 =
    "imag" components, cos/sin fed pre-transposed.
  - Scores are computed transposed: ST[sk, sq] = K @ Q^T. Softmax reduction
    over sk (partitions) is a ones-vector matmul; probabilities feed the PV
    matmul directly as rhs (ctxT = V^T @ expST) with no transposition.
  - ctxT is exactly the lhsT the wo matmul needs. 1/sqrt(head_dim) is folded
    into wq on the host. Softmax uses exp without max subtraction (scores are
    O(1) for this problem's input distribution) and multiplicative exp(mask)
    block patterns, deduplicated and usually resolved to skip/plain.
"""

import math
import numpy as np


def _rne11(x):
    """Round fp32 to the float32r grid (RNE at 11 mantissa bits)."""
    b = x.view(np.uint32).astype(np.uint64)
    bias = ((b >> 12) & 1) + 0x7FF
    return ((b + bias) >> 12 << 12).astype(np.uint32).view(np.float32)

P = 128          # SBUF partitions / head_dim / tile edge
S = 2048         # sequence length
D = 4096         # model dim
HD = 128         # head dim
N_HEADS = 32
N_KV = 8
N_CORES = 8
NH_LOC = N_HEADS // N_CORES   # 4 local Q heads
SG = 512         # score/free-dim group width (one PSUM bank of fp32)
NG = S // SG     # 4 q-position groups
KT = D // P      # 32 contraction tiles for projections
NSK = S // P     # 16 key tiles

_CACHE = {}


def _classify_mask(mask):
    """Classify each [P, SG] block of mask.T into skip / plain / masked.

    Returns (sk_lists, patterns):
      sk_lists[G] = list of (m, pat_idx_or_None) key-tiles to compute for
                    query group G, and patterns = [P, SG] multiplicative
                    exp(mask) blocks (deduped).
    """
    mt = np.ascontiguousarray(mask.T.astype(np.float32))
    patterns = []
    pat_idx = {}
    sk_lists = []
    for G in range(NG):
        lst = []
        for m in range(NSK):
            blk = mt[m * P:(m + 1) * P, G * SG:(G + 1) * SG]
            if np.all(np.isneginf(blk)):
                continue
            if np.all(blk == 0.0):
                lst.append((m, None))
                continue
            with np.errstate(over="ignore"):
                pat = np.exp(blk).astype(np.float32)
            key = pat.tobytes()
            if key not in pat_idx:
                pat_idx[key] = len(patterns)
                patterns.append(pat)
            lst.append((m, pat_idx[key]))
        sk_lists.append(lst)
    return sk_lists, patterns


def _build_program(sk_lists, n_pat):
    import concourse.tile as tile
    from concourse import bacc, mybir
    from concourse.masks import make_identity
    from contextlib import ExitStack

    f32 = mybir.dt.float32
    f32r = mybir.dt.float32r
    Exp = mybir.ActivationFunctionType.Exp

    nc = bacc.Bacc()
    XWB = SG + NH_LOC * HD        # one fused x|wq block: 1024 cols
    xw_d = nc.dram_tensor("xw", [P, NG * KT * XWB], f32r, kind="ExternalInput")
    wk_d = nc.dram_tensor("wk", [P, KT * HD], f32r, kind="ExternalInput")
    wv_d = nc.dram_tensor("wv", [P, KT * HD], f32r, kind="ExternalInput")
    wo_d = nc.dram_tensor("wo", [P, (D // SG) * NH_LOC * SG], f32r,
                          kind="ExternalInput")
    cs_d = nc.dram_tensor("cs", [P, S], f32, kind="ExternalInput")
    mb_d = None
    if n_pat:
        mb_d = nc.dram_tensor("mb", [n_pat, P, SG], f32r, kind="ExternalInput")
    out_d = nc.dram_tensor("out", [S, D], f32, kind="ExternalOutput")

    with ExitStack() as ctx:
        tc = ctx.enter_context(tile.TileContext(nc))
        consts = ctx.enter_context(tc.tile_pool(name="consts", bufs=1))
        kv = ctx.enter_context(tc.tile_pool(name="kv", bufs=1))
        xp = ctx.enter_context(tc.tile_pool(name="xp", bufs=4))
        qp = ctx.enter_context(tc.tile_pool(name="qp", bufs=1))
        rp = ctx.enter_context(tc.tile_pool(name="rp", bufs=4))
        ep = ctx.enter_context(tc.tile_pool(name="ep", bufs=4))
        sp = ctx.enter_context(tc.tile_pool(name="sp", bufs=4))
        cp = ctx.enter_context(tc.tile_pool(name="cp", bufs=8))
        ps = ctx.enter_context(tc.tile_pool(name="ps", bufs=8, space="PSUM"))

        # resident weights / constants (wq is streamed per-use; too big).
        # Consts ride the ACT DMA ring so the x|wq stream owns the SP ring.
        wk_sb = consts.tile([P, KT * HD], f32r)
        wv_sb = consts.tile([P, KT * HD], f32r)
        qtr = KT * HD // 4
        for i in range(4):
            nc.scalar.dma_start(wk_sb[:, i * qtr:(i + 1) * qtr],
                                wk_d[:, i * qtr:(i + 1) * qtr])
            nc.scalar.dma_start(wv_sb[:, i * qtr:(i + 1) * qtr],
                                wv_d[:, i * qtr:(i + 1) * qtr])
        cs_sb = consts.tile([P, S], f32)
        nc.scalar.dma_start(cs_sb[:], cs_d[:, :])
        mb_sb = None
        if n_pat:
            mb_sb = consts.tile([P, n_pat * SG], f32r)
            for i in range(n_pat):
                nc.scalar.dma_start(mb_sb[:, i * SG:(i + 1) * SG], mb_d[i])
        ones_f = consts.tile([P, 1], f32)
        nc.vector.memset(ones_f[:], 1.0)
        ones_col = consts.tile([P, 1], f32r)
        nc.vector.tensor_copy(ones_col[:], ones_f[:])
        ones_row = consts.tile([1, P], f32)
        nc.vector.memset(ones_row[:], 1.0)
        ident = consts.tile([P, P], f32)
        make_identity(nc, ident[:])

        # full-sequence KV + context accumulators
        kT_sb = kv.tile([P, S], f32r)                # [head_dim', s]
        v_sb = kv.tile([P, S], f32r)                # [s%P, (s//P)*HD + hd]
        ctx_sb = kv.tile([P, NH_LOC * S], f32r)       # [hd, h*S + sq]

        # pending per-head softmax finalization, emitted later so the PE
        # queue never stalls on the reciprocal chain (in-order engine)
        def finalize(fin):
            cacc, sacc, h, G0 = fin
            inv = sp.tile([1, SG], f32, tag="inv", bufs=2)
            nc.vector.reciprocal(inv[:], sacc[:])
            bc = ps.tile([P, SG], f32, tag="bank", bufs=8, name="bc")
            nc.tensor.matmul(bc[:], ones_row[:], inv[:], start=True, stop=True)
            bcs = sp.tile([P, SG], f32, tag="bcs", bufs=2)
            nc.vector.tensor_copy(bcs[:], bc[:])
            nc.vector.tensor_mul(
                ctx_sb[:, h * S + G0 * SG:h * S + (G0 + 1) * SG],
                cacc[:], bcs[:])

        pending = None
        for G in range(NG):
            gsl = slice(G * SG, (G + 1) * SG)
            # ---------------- phase A: projections for s-slice G -----------
            pq = [ps.tile([P, SG], f32, tag="bank", bufs=8, name=f"pq{_l}")
                  for _l in range(NH_LOC)]
            pk = ps.tile([P, SG], f32, tag="bank", bufs=8, name="pk")
            pv = ps.tile([P, SG], f32, tag="bank", bufs=8, name="pv")
            for k2 in range(KT // 2):
                xw = xp.tile([P, 2 * XWB], f32r, tag="xw", bufs=5, name="xw")
                blk = (G * KT + 2 * k2) * XWB
                ring = nc.sync if k2 % 2 == 0 else nc.scalar
                ring.dma_start(xw[:], xw_d[:, blk:blk + 2 * XWB])
                for k in (2 * k2, 2 * k2 + 1):
                    off = (k - 2 * k2) * XWB
                    xt = xw[:, off:off + SG]
                    st_k, sp_k = (k == 0), (k == KT - 1)
                    for l in range(NH_LOC):
                        nc.tensor.matmul(
                            pq[l][:],
                            xw[:, off + SG + l * HD:off + SG + (l + 1) * HD],
                            xt, start=st_k, stop=sp_k)
                    nc.tensor.matmul(pk[:], wk_sb[:, k * HD:(k + 1) * HD], xt,
                                     start=st_k, stop=sp_k)
                    nc.tensor.matmul(pv[:], wv_sb[:, k * HD:(k + 1) * HD], xt,
                                     start=st_k, stop=sp_k)

            if pending is not None:     # head 3 of the previous group
                finalize(pending)
                pending = None

            # RoPE (rows 0:64 real, 64:128 imag), PSUM -> SBUF.
            # Order q0 first then k: B(G, h=0) only needs q0 (+ kT for the
            # diagonal tiles, needed first only at G=0).
            qts = [None] * NH_LOC
            cos = cs_sb[0:64, gsl]
            sin = cs_sb[64:128, gsl]
            for l in (0, NH_LOC, 1, 2, 3):
                src = pq[l] if l < NH_LOC else pk
                if l < NH_LOC:
                    dst = qp.tile([P, SG], f32r, tag="qT", bufs=6, name="qT")
                    qts[l] = dst
                    dr, di = dst[0:64, :], dst[64:128, :]
                else:
                    dr, di = kT_sb[0:64, gsl], kT_sb[64:128, gsl]
                ta = rp.tile([64, SG], f32, tag="ropeA", bufs=2)
                tb = rp.tile([64, SG], f32, tag="ropeB", bufs=2)
                tcc = rp.tile([64, SG], f32, tag="ropeC", bufs=2)
                td = rp.tile([64, SG], f32, tag="ropeD", bufs=2)
                nc.vector.tensor_mul(ta[:], src[0:64, :], cos)
                nc.vector.tensor_mul(tcc[:], src[0:64, :], sin)
                nc.vector.tensor_mul(tb[:], src[64:128, :], sin)
                nc.vector.tensor_mul(td[:], src[64:128, :], cos)
                nc.vector.tensor_sub(dr, ta[:], tb[:])
                nc.vector.tensor_add(di, tcc[:], td[:])

            # vT -> v (PE transpose via identity)
            vt = sp.tile([P, SG], f32, tag="vtmp", bufs=2)
            nc.scalar.copy(vt[:], pv[:])
            for j in range(SG // P):
                ptr = ps.tile([P, P], f32, tag="bank", bufs=8, name="ptr")
                nc.tensor.transpose(ptr[:], vt[:, j * P:(j + 1) * P], ident[:])
                vdst = v_sb[:, (G * 4 + j) * HD:(G * 4 + j + 1) * HD]
                if j % 2:
                    nc.scalar.copy(vdst, ptr[:])
                else:
                    nc.vector.tensor_copy(vdst, ptr[:])

            # ---------------- phase B: attention for q-group G -------------
            for h in range(NH_LOC):
                cacc = ps.tile([P, SG], f32, tag="bank", bufs=8, name="cacc")
                sacc = ps.tile([1, SG], f32, tag="bank", bufs=8, name="sacc")
                n_sk = len(sk_lists[G])
                for idx, (m, pat) in enumerate(sk_lists[G]):
                    stp = ps.tile([P, SG], f32, tag="bank", bufs=8, name="stp")
                    nc.tensor.matmul(stp[:], kT_sb[:, m * P:(m + 1) * P],
                                     qts[h][:], start=True, stop=True)
                    ex = ep.tile([P, SG], f32r, tag="ex", bufs=3)
                    nc.scalar.activation(ex[:], stp[:], Exp)
                    if pat is not None:
                        nc.vector.tensor_mul(
                            ex[:], ex[:], mb_sb[:, pat * SG:(pat + 1) * SG])
                    st_a, sp_a = (idx == 0), (idx == n_sk - 1)
                    nc.tensor.matmul(cacc[:], v_sb[:, m * HD:(m + 1) * HD],
                                     ex[:], start=st_a, stop=sp_a)
                    nc.tensor.matmul(sacc[:], ones_col[:], ex[:],
                                     start=st_a, stop=sp_a)
                if pending is not None:
                    finalize(pending)
                pending = (cacc, sacc, h, G)
        finalize(pending)

        # ---------------- phase C: out = ctx @ wo (partial) ----------------
        for n in range(D // SG):
            wt = cp.tile([P, NH_LOC * SG], f32r, tag="wo", bufs=2, name="wot")
            nc.scalar.dma_start(
                wt[:], wo_d[:, n * NH_LOC * SG:(n + 1) * NH_LOC * SG])
            for m in range(NSK):
                po = ps.tile([P, SG], f32, tag="bank", bufs=8, name="po")
                for kk in range(NH_LOC):
                    nc.tensor.matmul(po[:],
                                     ctx_sb[:, kk * S + m * P:kk * S + (m + 1) * P],
                                     wt[:, kk * SG:(kk + 1) * SG],
                                     start=(kk == 0), stop=(kk == NH_LOC - 1))
                ot = cp.tile([P, SG], f32, tag="ot", bufs=3)
                if m % 2:
                    nc.scalar.copy(ot[:], po[:])
                else:
                    nc.vector.tensor_copy(ot[:], po[:])
                nc.sync.dma_start(out_d[m * P:(m + 1) * P, n * SG:(n + 1) * SG], ot[:])

    nc.compile()
    return nc


def _host_prep(x, wq, wk, wv, wo, freqs_cos, freqs_sin):
    """Build per-core input maps (all layouts pre-tiled for contiguous DMA)."""
    x = np.ascontiguousarray(np.asarray(x, dtype=np.float32).reshape(S, D))
    wq = np.asarray(wq, dtype=np.float32)
    wk = np.asarray(wk, dtype=np.float32)
    wv = np.asarray(wv, dtype=np.float32)
    wo = np.asarray(wo, dtype=np.float32)

    perm = np.concatenate([np.arange(0, HD, 2), np.arange(1, HD, 2)])
    scale = 1.0 / math.sqrt(HD)
    wq_p = (wq.reshape(D, N_HEADS, HD)[:, :, perm] * scale).astype(np.float32)
    wk_p = wk.reshape(D, N_KV, HD)[:, :, perm]

    # xT blocks: xtb[p, G, k, c] = x[G*SG + c, k*P + p]
    xtb = _rne11(np.ascontiguousarray(
        x.T.reshape(KT, P, NG, SG).transpose(1, 2, 0, 3)))   # [P, NG, KT, SG]
    cs = np.ascontiguousarray(
        np.concatenate([np.asarray(freqs_cos, np.float32).T,
                        np.asarray(freqs_sin, np.float32).T], axis=0))

    in_maps = []
    for c in range(N_CORES):
        wq_c = wq_p[:, 4 * c:4 * c + 4, :].reshape(D, NH_LOC * HD)
        wq_l = _rne11(np.ascontiguousarray(
            wq_c.reshape(KT, P, NH_LOC * HD).transpose(1, 0, 2)))  # [P, KT, 512]
        # fused x|wq stream: block (G, k) = [ xT(G,k) 512 | wq(k) 512 ]
        xw = np.empty((P, NG, KT, SG + NH_LOC * HD), np.float32)
        xw[:, :, :, :SG] = xtb
        xw[:, :, :, SG:] = wq_l[:, None, :, :]
        xw = np.ascontiguousarray(xw.reshape(P, NG * KT * (SG + NH_LOC * HD)))
        wk_c = wk_p[:, c, :]
        wk_l = np.ascontiguousarray(
            wk_c.reshape(KT, P, HD).transpose(1, 0, 2).reshape(P, KT * HD))
        wv_c = wv.reshape(D, N_KV, HD)[:, c, :]
        wv_l = np.ascontiguousarray(
            wv_c.reshape(KT, P, HD).transpose(1, 0, 2).reshape(P, KT * HD))
        wo_c = wo[4 * c * HD:(4 * c + 4) * HD, :]       # [512, D]
        # [P, n, kk, 512]: per dim-group n, the 4 head-chunk tiles adjacent
        wo_l = np.ascontiguousarray(
            wo_c.reshape(NH_LOC, P, D // SG, SG).transpose(1, 2, 0, 3)
            .reshape(P, (D // SG) * NH_LOC * SG))
        in_maps.append({"xw": xw, "wk": _rne11(wk_l),
                        "wv": _rne11(wv_l), "wo": _rne11(wo_l), "cs": cs})
    return in_maps


def _run(x, wq, wk, wv, wo, freqs_cos, freqs_sin, mask, start_pos, trace=False):
    assert int(start_pos) == 0
    sk_lists, patterns = _classify_mask(np.asarray(mask, dtype=np.float32))
    n_pat = len(patterns)
    fp = (tuple(tuple(lst) for lst in sk_lists), n_pat)

    if fp not in _CACHE:
        _CACHE[fp] = _build_program(sk_lists, n_pat)
    nc = _CACHE[fp]

    in_maps = _host_prep(x, wq, wk, wv, wo, freqs_cos, freqs_sin)
    if n_pat:
        mb = _rne11(np.ascontiguousarray(np.stack(patterns)))
        for m in in_maps:
            m["mb"] = mb

    from concourse.bass_utils import run_bass_kernel_spmd
    res = run_bass_kernel_spmd(nc, in_maps, list(range(N_CORES)), trace=trace)
    out = np.zeros((S, D), dtype=np.float32)
    for c in range(N_CORES):
        out += res.results[c]["out"]
    return out.reshape(1, S, D), res


def kernel(x, wq, wk, wv, wo, freqs_cos, freqs_sin, mask, start_pos):
    out, _ = _run(x, wq, wk, wv, wo, freqs_cos, freqs_sin, mask, start_pos)
    return out


# revision 21
# speedup vs baseline: 1.0110x; 1.0043x over previous
"""Trainium2 Bass kernel for Llama-style GQA attention (B=1, S=2048, D=4096,
32 Q heads / 8 KV heads, head_dim 128, RoPE, additive mask, causal-aware).

Sharding: 8-way tensor-parallel over heads. Core c computes Q heads 4c..4c+3
and KV head c end-to-end (projections + RoPE + attention + its rows of wo),
producing a partial [S, D] output; the host sums the 8 partials (the
all-reduce of the row-parallel wo).

Device layout strategy (all fp32):
  - Host feeds xT = x.T so Q/K projections produce qT/kT ([head_dim, s]) and
    the V projection produces vT, with zero on-device transposes of x.
  - RoPE's even/odd interleave is folded into a column permutation of wq/wk
    (scores are invariant under a shared permutation of q and k), making RoPE
    pure partition-aligned elementwise math: rows 0:64 = "real", 64:128 =
    "imag" components, cos/sin fed pre-transposed.
  - Scores are computed transposed: ST[sk, sq] = K @ Q^T. Softmax reduction
    over sk (partitions) is a ones-vector matmul; probabilities feed the PV
    matmul directly as rhs (ctxT = V^T @ expST) with no transposition.
  - ctxT is exactly the lhsT the wo matmul needs. 1/sqrt(head_dim) is folded
    into wq on the host. Softmax uses exp without max subtraction (scores are
    O(1) for this problem's input distribution) and multiplicative exp(mask)
    block patterns, deduplicated and usually resolved to skip/plain.
"""

import math
import numpy as np


def _rne11(x):
    """Round fp32 to the float32r grid (RNE at 11 mantissa bits)."""
    b = x.view(np.uint32).astype(np.uint64)
    bias = ((b >> 12) & 1) + 0x7FF
    return ((b + bias) >> 12 << 12).astype(np.uint32).view(np.float32)

P = 128          # SBUF partitions / head_dim / tile edge
S = 2048         # sequence length
D = 4096         # model dim
HD = 128         # head dim
N_HEADS = 32
N_KV = 8
N_CORES = 8
NH_LOC = N_HEADS // N_CORES   # 4 local Q heads
SG = 512         # score/free-dim group width (one PSUM bank of fp32)
NG = S // SG     # 4 q-position groups
KT = D // P      # 32 contraction tiles for projections
NSK = S // P     # 16 key tiles

_CACHE = {}


def _classify_mask(mask):
    """Classify each [P, SG] block of mask.T into skip / plain / masked.

    Returns (sk_lists, patterns):
      sk_lists[G] = list of (m, pat_idx_or_None) key-tiles to compute for
                    query group G, and patterns = [P, SG] multiplicative
                    exp(mask) blocks (deduped).
    """
    mt = np.ascontiguousarray(mask.T.astype(np.float32))
    patterns = []
    pat_idx = {}
    sk_lists = []
    for G in range(NG):
        lst = []
        for m in range(NSK):
            blk = mt[m * P:(m + 1) * P, G * SG:(G + 1) * SG]
            if np.all(np.isneginf(blk)):
                continue
            if np.all(blk == 0.0):
                lst.append((m, None))
                continue
            with np.errstate(over="ignore"):
                pat = np.exp(blk).astype(np.float32)
            key = pat.tobytes()
            if key not in pat_idx:
                pat_idx[key] = len(patterns)
                patterns.append(pat)
            lst.append((m, pat_idx[key]))
        sk_lists.append(lst)
    return sk_lists, patterns


def _build_program(sk_lists, n_pat):
    import concourse.tile as tile
    from concourse import bacc, mybir
    from concourse.masks import make_identity
    from contextlib import ExitStack

    f32 = mybir.dt.float32
    f32r = mybir.dt.float32r
    Exp = mybir.ActivationFunctionType.Exp

    nc = bacc.Bacc()
    XWB = SG + NH_LOC * HD        # one fused x|wq block: 1024 cols
    xw_d = nc.dram_tensor("xw", [P, NG * KT * XWB], f32r, kind="ExternalInput")
    wk_d = nc.dram_tensor("wk", [P, KT * HD], f32r, kind="ExternalInput")
    wv_d = nc.dram_tensor("wv", [P, KT * HD], f32r, kind="ExternalInput")
    wo_d = nc.dram_tensor("wo", [P, (D // SG) * NH_LOC * SG], f32r,
                          kind="ExternalInput")
    cs_d = nc.dram_tensor("cs", [P, S], f32, kind="ExternalInput")
    mb_d = None
    if n_pat:
        mb_d = nc.dram_tensor("mb", [n_pat, P, SG], f32r, kind="ExternalInput")
    out_d = nc.dram_tensor("out", [S, D], f32, kind="ExternalOutput")

    with ExitStack() as ctx:
        tc = ctx.enter_context(tile.TileContext(nc))
        consts = ctx.enter_context(tc.tile_pool(name="consts", bufs=1))
        kv = ctx.enter_context(tc.tile_pool(name="kv", bufs=1))
        xp = ctx.enter_context(tc.tile_pool(name="xp", bufs=4))
        qp = ctx.enter_context(tc.tile_pool(name="qp", bufs=1))
        rp = ctx.enter_context(tc.tile_pool(name="rp", bufs=4))
        ep = ctx.enter_context(tc.tile_pool(name="ep", bufs=4))
        sp = ctx.enter_context(tc.tile_pool(name="sp", bufs=4))
        cp = ctx.enter_context(tc.tile_pool(name="cp", bufs=8))
        ps = ctx.enter_context(tc.tile_pool(name="ps", bufs=8, space="PSUM"))

        # resident weights / constants (wq is streamed per-use; too big).
        # Consts ride the ACT DMA ring so the x|wq stream owns the SP ring.
        wk_sb = consts.tile([P, KT * HD], f32r)
        wv_sb = consts.tile([P, KT * HD], f32r)
        qtr = KT * HD // 4
        for i in range(4):
            nc.scalar.dma_start(wk_sb[:, i * qtr:(i + 1) * qtr],
                                wk_d[:, i * qtr:(i + 1) * qtr])
            nc.scalar.dma_start(wv_sb[:, i * qtr:(i + 1) * qtr],
                                wv_d[:, i * qtr:(i + 1) * qtr])
        cs_sb = consts.tile([P, S], f32)
        nc.scalar.dma_start(cs_sb[:], cs_d[:, :])
        mb_sb = None
        if n_pat:
            mb_sb = consts.tile([P, n_pat * SG], f32r)
            for i in range(n_pat):
                nc.scalar.dma_start(mb_sb[:, i * SG:(i + 1) * SG], mb_d[i])
        ones_f = consts.tile([P, 1], f32)
        nc.vector.memset(ones_f[:], 1.0)
        ones_col = consts.tile([P, 1], f32r)
        nc.vector.tensor_copy(ones_col[:], ones_f[:])
        ones_row = consts.tile([1, P], f32)
        nc.vector.memset(ones_row[:], 1.0)
        ident = consts.tile([P, P], f32)
        make_identity(nc, ident[:])

        # full-sequence KV + context accumulators
        kT_sb = kv.tile([P, S], f32r)                # [head_dim', s]
        v_sb = kv.tile([P, S], f32r)                # [s%P, (s//P)*HD + hd]
        ctx_sb = kv.tile([P, NH_LOC * S], f32r)       # [hd, h*S + sq]

        # pending per-head softmax finalization, emitted later so the PE
        # queue never stalls on the reciprocal chain (in-order engine)
        def finalize(fin):
            cacc, sacc, h, G0 = fin
            inv = sp.tile([1, SG], f32, tag="inv", bufs=2)
            nc.vector.reciprocal(inv[:], sacc[:])
            bc = ps.tile([P, SG], f32, tag="bank", bufs=8, name="bc")
            nc.tensor.matmul(bc[:], ones_row[:], inv[:], start=True, stop=True)
            bcs = sp.tile([P, SG], f32, tag="bcs", bufs=2)
            nc.vector.tensor_copy(bcs[:], bc[:])
            nc.vector.tensor_mul(
                ctx_sb[:, h * S + G0 * SG:h * S + (G0 + 1) * SG],
                cacc[:], bcs[:])

        pending = None
        for G in range(NG):
            gsl = slice(G * SG, (G + 1) * SG)
            # ---------------- phase A: projections for s-slice G -----------
            pq = [ps.tile([P, SG], f32, tag="bank", bufs=8, name=f"pq{_l}")
                  for _l in range(NH_LOC)]
            pk = ps.tile([P, SG], f32, tag="bank", bufs=8, name="pk")
            pv = ps.tile([P, SG], f32, tag="bank", bufs=8, name="pv")
            for k2 in range(KT // 2):
                xw = xp.tile([P, 2 * XWB], f32r, tag="xw", bufs=5, name="xw")
                blk = (G * KT + 2 * k2) * XWB
                nc.sync.dma_start(xw[:], xw_d[:, blk:blk + 2 * XWB])
                for k in (2 * k2, 2 * k2 + 1):
                    off = (k - 2 * k2) * XWB
                    xt = xw[:, off:off + SG]
                    st_k, sp_k = (k == 0), (k == KT - 1)
                    for l in range(NH_LOC):
                        nc.tensor.matmul(
                            pq[l][:],
                            xw[:, off + SG + l * HD:off + SG + (l + 1) * HD],
                            xt, start=st_k, stop=sp_k)
                    nc.tensor.matmul(pk[:], wk_sb[:, k * HD:(k + 1) * HD], xt,
                                     start=st_k, stop=sp_k)
                    nc.tensor.matmul(pv[:], wv_sb[:, k * HD:(k + 1) * HD], xt,
                                     start=st_k, stop=sp_k)

            if pending is not None:     # head 3 of the previous group
                finalize(pending)
                pending = None

            # RoPE (rows 0:64 real, 64:128 imag), PSUM -> SBUF.
            # Order q0 first then k: B(G, h=0) only needs q0 (+ kT for the
            # diagonal tiles, needed first only at G=0).
            qts = [None] * NH_LOC
            cos = cs_sb[0:64, gsl]
            sin = cs_sb[64:128, gsl]
            # k's rope is only needed by the diagonal tiles, which come last
            # within each head for G>0 - emit it last there so q1..q3 are
            # ready sooner (B(G,h) stalls on qts[h] otherwise). G=0's tiles
            # are all diagonal, so k goes right after q0.
            rope_order = (0, NH_LOC, 1, 2, 3) if G == 0 else (0, 1, 2, 3, NH_LOC)
            for l in rope_order:
                src = pq[l] if l < NH_LOC else pk
                if l < NH_LOC:
                    dst = qp.tile([P, SG], f32r, tag="qT", bufs=6, name="qT")
                    qts[l] = dst
                    dr, di = dst[0:64, :], dst[64:128, :]
                else:
                    dr, di = kT_sb[0:64, gsl], kT_sb[64:128, gsl]
                ta = rp.tile([64, SG], f32, tag="ropeA", bufs=2)
                tb = rp.tile([64, SG], f32, tag="ropeB", bufs=2)
                tcc = rp.tile([64, SG], f32, tag="ropeC", bufs=2)
                td = rp.tile([64, SG], f32, tag="ropeD", bufs=2)
                nc.vector.tensor_mul(ta[:], src[0:64, :], cos)
                nc.vector.tensor_mul(tcc[:], src[0:64, :], sin)
                nc.vector.tensor_mul(tb[:], src[64:128, :], sin)
                nc.vector.tensor_mul(td[:], src[64:128, :], cos)
                nc.vector.tensor_sub(dr, ta[:], tb[:])
                nc.vector.tensor_add(di, tcc[:], td[:])

            # vT -> v (PE transpose via identity)
            vt = sp.tile([P, SG], f32, tag="vtmp", bufs=2)
            nc.scalar.copy(vt[:], pv[:])
            for j in range(SG // P):
                ptr = ps.tile([P, P], f32, tag="bank", bufs=8, name="ptr")
                nc.tensor.transpose(ptr[:], vt[:, j * P:(j + 1) * P], ident[:])
                vdst = v_sb[:, (G * 4 + j) * HD:(G * 4 + j + 1) * HD]
                if j % 2:
                    nc.scalar.copy(vdst, ptr[:])
                else:
                    nc.vector.tensor_copy(vdst, ptr[:])

            # ---------------- phase B: attention for q-group G -------------
            for h in range(NH_LOC):
                cacc = ps.tile([P, SG], f32, tag="bank", bufs=8, name="cacc")
                sacc = ps.tile([1, SG], f32, tag="bank", bufs=8, name="sacc")
                n_sk = len(sk_lists[G])
                for idx, (m, pat) in enumerate(sk_lists[G]):
                    stp = ps.tile([P, SG], f32, tag="bank", bufs=8, name="stp")
                    nc.tensor.matmul(stp[:], kT_sb[:, m * P:(m + 1) * P],
                                     qts[h][:], start=True, stop=True)
                    ex = ep.tile([P, SG], f32r, tag="ex", bufs=3)
                    nc.scalar.activation(ex[:], stp[:], Exp)
                    if pat is not None:
                        # late heads' masked tiles sit right where the next
                        # group's rope needs the DVE - use GPSIMD there
                        eng = nc.gpsimd if h >= 2 else nc.vector
                        eng.tensor_mul(
                            ex[:], ex[:], mb_sb[:, pat * SG:(pat + 1) * SG])
                    st_a, sp_a = (idx == 0), (idx == n_sk - 1)
                    nc.tensor.matmul(cacc[:], v_sb[:, m * HD:(m + 1) * HD],
                                     ex[:], start=st_a, stop=sp_a)
                    nc.tensor.matmul(sacc[:], ones_col[:], ex[:],
                                     start=st_a, stop=sp_a)
                if pending is not None:
                    finalize(pending)
                pending = (cacc, sacc, h, G)
        finalize(pending)

        # ---------------- phase C: out = ctx @ wo (partial) ----------------
        for n in range(D // SG):
            wt = cp.tile([P, NH_LOC * SG], f32r, tag="wo", bufs=2, name="wot")
            nc.scalar.dma_start(
                wt[:], wo_d[:, n * NH_LOC * SG:(n + 1) * NH_LOC * SG])
            for m in range(NSK):
                po = ps.tile([P, SG], f32, tag="bank", bufs=8, name="po")
                for kk in range(NH_LOC):
                    nc.tensor.matmul(po[:],
                                     ctx_sb[:, kk * S + m * P:kk * S + (m + 1) * P],
                                     wt[:, kk * SG:(kk + 1) * SG],
                                     start=(kk == 0), stop=(kk == NH_LOC - 1))
                ot = cp.tile([P, SG], f32, tag="ot", bufs=3)
                if m % 2:
                    nc.scalar.copy(ot[:], po[:])
                else:
                    nc.vector.tensor_copy(ot[:], po[:])
                nc.sync.dma_start(out_d[m * P:(m + 1) * P, n * SG:(n + 1) * SG], ot[:])

    nc.compile()
    return nc


def _host_prep(x, wq, wk, wv, wo, freqs_cos, freqs_sin):
    """Build per-core input maps (all layouts pre-tiled for contiguous DMA)."""
    x = np.ascontiguousarray(np.asarray(x, dtype=np.float32).reshape(S, D))
    wq = np.asarray(wq, dtype=np.float32)
    wk = np.asarray(wk, dtype=np.float32)
    wv = np.asarray(wv, dtype=np.float32)
    wo = np.asarray(wo, dtype=np.float32)

    perm = np.concatenate([np.arange(0, HD, 2), np.arange(1, HD, 2)])
    scale = 1.0 / math.sqrt(HD)
    wq_p = (wq.reshape(D, N_HEADS, HD)[:, :, perm] * scale).astype(np.float32)
    wk_p = wk.reshape(D, N_KV, HD)[:, :, perm]

    # xT blocks: xtb[p, G, k, c] = x[G*SG + c, k*P + p]
    xtb = _rne11(np.ascontiguousarray(
        x.T.reshape(KT, P, NG, SG).transpose(1, 2, 0, 3)))   # [P, NG, KT, SG]
    cs = np.ascontiguousarray(
        np.concatenate([np.asarray(freqs_cos, np.float32).T,
                        np.asarray(freqs_sin, np.float32).T], axis=0))

    in_maps = []
    for c in range(N_CORES):
        wq_c = wq_p[:, 4 * c:4 * c + 4, :].reshape(D, NH_LOC * HD)
        wq_l = _rne11(np.ascontiguousarray(
            wq_c.reshape(KT, P, NH_LOC * HD).transpose(1, 0, 2)))  # [P, KT, 512]
        # fused x|wq stream: block (G, k) = [ xT(G,k) 512 | wq(k) 512 ]
        xw = np.empty((P, NG, KT, SG + NH_LOC * HD), np.float32)
        xw[:, :, :, :SG] = xtb
        xw[:, :, :, SG:] = wq_l[:, None, :, :]
        xw = np.ascontiguousarray(xw.reshape(P, NG * KT * (SG + NH_LOC * HD)))
        wk_c = wk_p[:, c, :]
        wk_l = np.ascontiguousarray(
            wk_c.reshape(KT, P, HD).transpose(1, 0, 2).reshape(P, KT * HD))
        wv_c = wv.reshape(D, N_KV, HD)[:, c, :]
        wv_l = np.ascontiguousarray(
            wv_c.reshape(KT, P, HD).transpose(1, 0, 2).reshape(P, KT * HD))
        wo_c = wo[4 * c * HD:(4 * c + 4) * HD, :]       # [512, D]
        # [P, n, kk, 512]: per dim-group n, the 4 head-chunk tiles adjacent
        wo_l = np.ascontiguousarray(
            wo_c.reshape(NH_LOC, P, D // SG, SG).transpose(1, 2, 0, 3)
            .reshape(P, (D // SG) * NH_LOC * SG))
        in_maps.append({"xw": xw, "wk": _rne11(wk_l),
                        "wv": _rne11(wv_l), "wo": _rne11(wo_l), "cs": cs})
    return in_maps


def _run(x, wq, wk, wv, wo, freqs_cos, freqs_sin, mask, start_pos, trace=False):
    assert int(start_pos) == 0
    sk_lists, patterns = _classify_mask(np.asarray(mask, dtype=np.float32))
    n_pat = len(patterns)
    fp = (tuple(tuple(lst) for lst in sk_lists), n_pat)

    if fp not in _CACHE:
        _CACHE[fp] = _build_program(sk_lists, n_pat)
    nc = _CACHE[fp]

    in_maps = _host_prep(x, wq, wk, wv, wo, freqs_cos, freqs_sin)
    if n_pat:
        mb = _rne11(np.ascontiguousarray(np.stack(patterns)))
        for m in in_maps:
            m["mb"] = mb

    from concourse.bass_utils import run_bass_kernel_spmd
    res = run_bass_kernel_spmd(nc, in_maps, list(range(N_CORES)), trace=trace)
    out = np.zeros((S, D), dtype=np.float32)
    for c in range(N_CORES):
        out += res.results[c]["out"]
    return out.reshape(1, S, D), res


def kernel(x, wq, wk, wv, wo, freqs_cos, freqs_sin, mask, start_pos):
    out, _ = _run(x, wq, wk, wv, wo, freqs_cos, freqs_sin, mask, start_pos)
    return out


# revision 22
# speedup vs baseline: 1.0399x; 1.0285x over previous
"""Trainium2 Bass kernel for Llama-style GQA attention (B=1, S=2048, D=4096,
32 Q heads / 8 KV heads, head_dim 128, RoPE, additive mask, causal-aware).

Sharding: 8-way tensor-parallel over heads. Core c computes Q heads 4c..4c+3
and KV head c end-to-end (projections + RoPE + attention + its rows of wo),
producing a partial [S, D] output; the host sums the 8 partials (the
all-reduce of the row-parallel wo).

Device layout strategy (all fp32):
  - Host feeds xT = x.T so Q/K projections produce qT/kT ([head_dim, s]) and
    the V projection produces vT, with zero on-device transposes of x.
  - RoPE's even/odd interleave is folded into a column permutation of wq/wk
    (scores are invariant under a shared permutation of q and k), making RoPE
    pure partition-aligned elementwise math: rows 0:64 = "real", 64:128 =
    "imag" components, cos/sin fed pre-transposed.
  - Scores are computed transposed: ST[sk, sq] = K @ Q^T. Softmax reduction
    over sk (partitions) is a ones-vector matmul; probabilities feed the PV
    matmul directly as rhs (ctxT = V^T @ expST) with no transposition.
  - ctxT is exactly the lhsT the wo matmul needs. 1/sqrt(head_dim) is folded
    into wq on the host. Softmax uses exp without max subtraction (scores are
    O(1) for this problem's input distribution) and multiplicative exp(mask)
    block patterns, deduplicated and usually resolved to skip/plain.
"""

import math
import numpy as np


def _rne11(x):
    """Round fp32 to the float32r grid (RNE at 11 mantissa bits)."""
    b = x.view(np.uint32).astype(np.uint64)
    bias = ((b >> 12) & 1) + 0x7FF
    return ((b + bias) >> 12 << 12).astype(np.uint32).view(np.float32)

P = 128          # SBUF partitions / head_dim / tile edge
S = 2048         # sequence length
D = 4096         # model dim
HD = 128         # head dim
N_HEADS = 32
N_KV = 8
N_CORES = 8
NH_LOC = N_HEADS // N_CORES   # 4 local Q heads
SG = 512         # score/free-dim group width (one PSUM bank of fp32)
NG = S // SG     # 4 q-position groups
KT = D // P      # 32 contraction tiles for projections
NSK = S // P     # 16 key tiles

_CACHE = {}


def _classify_mask(mask):
    """Classify each [P, SG] block of mask.T into skip / plain / masked.

    Returns (sk_lists, patterns):
      sk_lists[G] = list of (m, pat_idx_or_None) key-tiles to compute for
                    query group G, and patterns = [P, SG] multiplicative
                    exp(mask) blocks (deduped).
    """
    mt = np.ascontiguousarray(mask.T.astype(np.float32))
    patterns = []
    pat_idx = {}
    sk_lists = []
    for G in range(NG):
        lst = []
        for m in range(NSK):
            blk = mt[m * P:(m + 1) * P, G * SG:(G + 1) * SG]
            if np.all(np.isneginf(blk)):
                continue
            if np.all(blk == 0.0):
                lst.append((m, None))
                continue
            with np.errstate(over="ignore"):
                pat = np.exp(blk).astype(np.float32)
            key = pat.tobytes()
            if key not in pat_idx:
                pat_idx[key] = len(patterns)
                patterns.append(pat)
            lst.append((m, pat_idx[key]))
        sk_lists.append(lst)
    return sk_lists, patterns


def _build_program(sk_lists, n_pat):
    import concourse.tile as tile
    from concourse import bacc, mybir
    from concourse.masks import make_identity
    from contextlib import ExitStack

    f32 = mybir.dt.float32
    f32r = mybir.dt.float32r
    Exp = mybir.ActivationFunctionType.Exp

    nc = bacc.Bacc()
    XWB = SG + NH_LOC * HD        # one fused x|wq block: 1024 cols
    xw_d = nc.dram_tensor("xw", [P, NG * KT * XWB], f32r, kind="ExternalInput")
    wk_d = nc.dram_tensor("wk", [P, KT * HD], f32r, kind="ExternalInput")
    wv_d = nc.dram_tensor("wv", [P, KT * HD], f32r, kind="ExternalInput")
    wo_d = nc.dram_tensor("wo", [P, (D // SG) * NH_LOC * SG], f32r,
                          kind="ExternalInput")
    cs_d = nc.dram_tensor("cs", [P, S], f32, kind="ExternalInput")
    mb_d = None
    if n_pat:
        mb_d = nc.dram_tensor("mb", [n_pat, P, SG], f32r, kind="ExternalInput")
    out_d = nc.dram_tensor("out", [S, D], f32, kind="ExternalOutput")

    with ExitStack() as ctx:
        tc = ctx.enter_context(tile.TileContext(nc))
        consts = ctx.enter_context(tc.tile_pool(name="consts", bufs=1))
        kv = ctx.enter_context(tc.tile_pool(name="kv", bufs=1))
        xp = ctx.enter_context(tc.tile_pool(name="xp", bufs=4))
        qp = ctx.enter_context(tc.tile_pool(name="qp", bufs=1))
        rp = ctx.enter_context(tc.tile_pool(name="rp", bufs=4))
        ep = ctx.enter_context(tc.tile_pool(name="ep", bufs=4))
        sp = ctx.enter_context(tc.tile_pool(name="sp", bufs=4))
        cp = ctx.enter_context(tc.tile_pool(name="cp", bufs=8))
        ps = ctx.enter_context(tc.tile_pool(name="ps", bufs=8, space="PSUM"))

        # resident weights / constants (wq is streamed per-use; too big).
        # Consts ride the ACT DMA ring so the x|wq stream owns the SP ring.
        wk_sb = consts.tile([P, KT * HD], f32r)
        wv_sb = consts.tile([P, KT * HD], f32r)
        qtr = KT * HD // 4
        for i in range(4):
            nc.scalar.dma_start(wk_sb[:, i * qtr:(i + 1) * qtr],
                                wk_d[:, i * qtr:(i + 1) * qtr])
            nc.scalar.dma_start(wv_sb[:, i * qtr:(i + 1) * qtr],
                                wv_d[:, i * qtr:(i + 1) * qtr])
        cs_sb = consts.tile([P, S], f32)
        nc.scalar.dma_start(cs_sb[:], cs_d[:, :])
        mb_sb = None
        if n_pat:
            mb_sb = consts.tile([P, n_pat * SG], f32r)
            for i in range(n_pat):
                nc.scalar.dma_start(mb_sb[:, i * SG:(i + 1) * SG], mb_d[i])
        ones_f = consts.tile([P, 1], f32)
        nc.vector.memset(ones_f[:], 1.0)
        ones_col = consts.tile([P, 1], f32r)
        nc.vector.tensor_copy(ones_col[:], ones_f[:])
        ones_row = consts.tile([1, P], f32)
        nc.vector.memset(ones_row[:], 1.0)
        ident = consts.tile([P, P], f32)
        make_identity(nc, ident[:])

        # full-sequence KV + context accumulators
        kT_sb = kv.tile([P, S], f32r)                # [head_dim', s]
        v_sb = kv.tile([P, S], f32r)                # [s%P, (s//P)*HD + hd]
        ctx_sb = kv.tile([P, NH_LOC * S], f32r)       # [hd, h*S + sq]

        # pending per-head softmax finalization, emitted later so the PE
        # queue never stalls on the reciprocal chain (in-order engine)
        def finalize(fin):
            cacc, sacc, h, G0 = fin
            inv = sp.tile([1, SG], f32, tag="inv", bufs=2)
            nc.vector.reciprocal(inv[:], sacc[:])
            bc = ps.tile([P, SG], f32, tag="bank", bufs=8, name="bc")
            nc.tensor.matmul(bc[:], ones_row[:], inv[:], start=True, stop=True)
            bcs = sp.tile([P, SG], f32, tag="bcs", bufs=2)
            nc.vector.tensor_copy(bcs[:], bc[:])
            nc.vector.tensor_mul(
                ctx_sb[:, h * S + G0 * SG:h * S + (G0 + 1) * SG],
                cacc[:], bcs[:])

        pending = None
        for G in range(NG):
            gsl = slice(G * SG, (G + 1) * SG)
            # ---------------- phase A: projections for s-slice G -----------
            pq = [ps.tile([P, SG], f32, tag="bank", bufs=8, name=f"pq{_l}")
                  for _l in range(NH_LOC)]
            pk = ps.tile([P, SG], f32, tag="bank", bufs=8, name="pk")
            pv = ps.tile([P, SG], f32, tag="bank", bufs=8, name="pv")
            for k2 in range(KT // 2):
                xw = xp.tile([P, 2 * XWB], f32r, tag="xw", bufs=5, name="xw")
                blk = (G * KT + 2 * k2) * XWB
                nc.sync.dma_start(xw[:], xw_d[:, blk:blk + 2 * XWB])
                for k in (2 * k2, 2 * k2 + 1):
                    off = (k - 2 * k2) * XWB
                    xt = xw[:, off:off + SG]
                    st_k, sp_k = (k == 0), (k == KT - 1)
                    for l in range(NH_LOC):
                        nc.tensor.matmul(
                            pq[l][:],
                            xw[:, off + SG + l * HD:off + SG + (l + 1) * HD],
                            xt, start=st_k, stop=sp_k)
                    nc.tensor.matmul(pk[:], wk_sb[:, k * HD:(k + 1) * HD], xt,
                                     start=st_k, stop=sp_k)
                    nc.tensor.matmul(pv[:], wv_sb[:, k * HD:(k + 1) * HD], xt,
                                     start=st_k, stop=sp_k)

            if pending is not None:     # head 3 of the previous group
                finalize(pending)
                pending = None

            # RoPE (rows 0:64 real, 64:128 imag), PSUM -> SBUF.
            # Order q0 first then k: B(G, h=0) only needs q0 (+ kT for the
            # diagonal tiles, needed first only at G=0).
            qts = [None] * NH_LOC
            cos = cs_sb[0:64, gsl]
            sin = cs_sb[64:128, gsl]
            for l in (0, NH_LOC, 1, 2, 3):
                src = pq[l] if l < NH_LOC else pk
                if l < NH_LOC:
                    dst = qp.tile([P, SG], f32r, tag="qT", bufs=6, name="qT")
                    qts[l] = dst
                    dr, di = dst[0:64, :], dst[64:128, :]
                else:
                    dr, di = kT_sb[0:64, gsl], kT_sb[64:128, gsl]
                ta = rp.tile([64, SG], f32, tag="ropeA", bufs=2)
                tb = rp.tile([64, SG], f32, tag="ropeB", bufs=2)
                tcc = rp.tile([64, SG], f32, tag="ropeC", bufs=2)
                td = rp.tile([64, SG], f32, tag="ropeD", bufs=2)
                nc.vector.tensor_mul(ta[:], src[0:64, :], cos)
                nc.vector.tensor_mul(tcc[:], src[0:64, :], sin)
                nc.vector.tensor_mul(tb[:], src[64:128, :], sin)
                nc.vector.tensor_mul(td[:], src[64:128, :], cos)
                nc.vector.tensor_sub(dr, ta[:], tb[:])
                nc.vector.tensor_add(di, tcc[:], td[:])

            # vT -> v (PE transpose via identity)
            vt = sp.tile([P, SG], f32, tag="vtmp", bufs=2)
            nc.scalar.copy(vt[:], pv[:])
            for j in range(SG // P):
                ptr = ps.tile([P, P], f32, tag="bank", bufs=8, name="ptr")
                nc.tensor.transpose(ptr[:], vt[:, j * P:(j + 1) * P], ident[:])
                vdst = v_sb[:, (G * 4 + j) * HD:(G * 4 + j + 1) * HD]
                if j % 2:
                    nc.scalar.copy(vdst, ptr[:])
                else:
                    nc.vector.tensor_copy(vdst, ptr[:])

            # ---------------- phase B: attention for q-group G -------------
            for h in range(NH_LOC):
                cacc = ps.tile([P, SG], f32, tag="bank", bufs=8, name="cacc")
                sacc = ps.tile([1, SG], f32, tag="bank", bufs=8, name="sacc")
                n_sk = len(sk_lists[G])
                for idx, (m, pat) in enumerate(sk_lists[G]):
                    stp = ps.tile([P, SG], f32, tag="bank", bufs=8, name="stp")
                    nc.tensor.matmul(stp[:], kT_sb[:, m * P:(m + 1) * P],
                                     qts[h][:], start=True, stop=True)
                    ex = ep.tile([P, SG], f32r, tag="ex", bufs=3)
                    nc.scalar.activation(ex[:], stp[:], Exp)
                    if pat is not None:
                        nc.vector.tensor_mul(
                            ex[:], ex[:], mb_sb[:, pat * SG:(pat + 1) * SG])
                    st_a, sp_a = (idx == 0), (idx == n_sk - 1)
                    nc.tensor.matmul(cacc[:], v_sb[:, m * HD:(m + 1) * HD],
                                     ex[:], start=st_a, stop=sp_a)
                    nc.tensor.matmul(sacc[:], ones_col[:], ex[:],
                                     start=st_a, stop=sp_a)
                if pending is not None:
                    finalize(pending)
                pending = (cacc, sacc, h, G)
        finalize(pending)

        # ---------------- phase C: out = ctx @ wo (partial) ----------------
        for n in range(D // SG):
            wt = cp.tile([P, NH_LOC * SG], f32r, tag="wo", bufs=2, name="wot")
            nc.scalar.dma_start(
                wt[:], wo_d[:, n * NH_LOC * SG:(n + 1) * NH_LOC * SG])
            for m in range(NSK):
                po = ps.tile([P, SG], f32, tag="bank", bufs=8, name="po")
                for kk in range(NH_LOC):
                    nc.tensor.matmul(po[:],
                                     ctx_sb[:, kk * S + m * P:kk * S + (m + 1) * P],
                                     wt[:, kk * SG:(kk + 1) * SG],
                                     start=(kk == 0), stop=(kk == NH_LOC - 1))
                ot = cp.tile([P, SG], f32, tag="ot", bufs=3)
                if m % 2:
                    nc.scalar.copy(ot[:], po[:])
                else:
                    nc.vector.tensor_copy(ot[:], po[:])
                nc.sync.dma_start(out_d[m * P:(m + 1) * P, n * SG:(n + 1) * SG], ot[:])

    nc.compile()
    return nc


def _host_prep(x, wq, wk, wv, wo, freqs_cos, freqs_sin):
    """Build per-core input maps (all layouts pre-tiled for contiguous DMA)."""
    x = np.ascontiguousarray(np.asarray(x, dtype=np.float32).reshape(S, D))
    wq = np.asarray(wq, dtype=np.float32)
    wk = np.asarray(wk, dtype=np.float32)
    wv = np.asarray(wv, dtype=np.float32)
    wo = np.asarray(wo, dtype=np.float32)

    perm = np.concatenate([np.arange(0, HD, 2), np.arange(1, HD, 2)])
    scale = 1.0 / math.sqrt(HD)
    wq_p = (wq.reshape(D, N_HEADS, HD)[:, :, perm] * scale).astype(np.float32)
    wk_p = wk.reshape(D, N_KV, HD)[:, :, perm]

    # xT blocks: xtb[p, G, k, c] = x[G*SG + c, k*P + p]
    xtb = _rne11(np.ascontiguousarray(
        x.T.reshape(KT, P, NG, SG).transpose(1, 2, 0, 3)))   # [P, NG, KT, SG]
    cs = np.ascontiguousarray(
        np.concatenate([np.asarray(freqs_cos, np.float32).T,
                        np.asarray(freqs_sin, np.float32).T], axis=0))

    in_maps = []
    for c in range(N_CORES):
        wq_c = wq_p[:, 4 * c:4 * c + 4, :].reshape(D, NH_LOC * HD)
        wq_l = _rne11(np.ascontiguousarray(
            wq_c.reshape(KT, P, NH_LOC * HD).transpose(1, 0, 2)))  # [P, KT, 512]
        # fused x|wq stream: block (G, k) = [ xT(G,k) 512 | wq(k) 512 ]
        xw = np.empty((P, NG, KT, SG + NH_LOC * HD), np.float32)
        xw[:, :, :, :SG] = xtb
        xw[:, :, :, SG:] = wq_l[:, None, :, :]
        xw = np.ascontiguousarray(xw.reshape(P, NG * KT * (SG + NH_LOC * HD)))
        wk_c = wk_p[:, c, :]
        wk_l = np.ascontiguousarray(
            wk_c.reshape(KT, P, HD).transpose(1, 0, 2).reshape(P, KT * HD))
        wv_c = wv.reshape(D, N_KV, HD)[:, c, :]
        wv_l = np.ascontiguousarray(
            wv_c.reshape(KT, P, HD).transpose(1, 0, 2).reshape(P, KT * HD))
        wo_c = wo[4 * c * HD:(4 * c + 4) * HD, :]       # [512, D]
        # [P, n, kk, 512]: per dim-group n, the 4 head-chunk tiles adjacent
        wo_l = np.ascontiguousarray(
            wo_c.reshape(NH_LOC, P, D // SG, SG).transpose(1, 2, 0, 3)
            .reshape(P, (D // SG) * NH_LOC * SG))
        in_maps.append({"xw": xw, "wk": _rne11(wk_l),
                        "wv": _rne11(wv_l), "wo": _rne11(wo_l), "cs": cs})
    return in_maps


def _run(x, wq, wk, wv, wo, freqs_cos, freqs_sin, mask, start_pos, trace=False):
    assert int(start_pos) == 0
    sk_lists, patterns = _classify_mask(np.asarray(mask, dtype=np.float32))
    n_pat = len(patterns)
    fp = (tuple(tuple(lst) for lst in sk_lists), n_pat)

    if fp not in _CACHE:
        _CACHE[fp] = _build_program(sk_lists, n_pat)
    nc = _CACHE[fp]

    in_maps = _host_prep(x, wq, wk, wv, wo, freqs_cos, freqs_sin)
    if n_pat:
        mb = _rne11(np.ascontiguousarray(np.stack(patterns)))
        for m in in_maps:
            m["mb"] = mb

    from concourse.bass_utils import run_bass_kernel_spmd
    res = run_bass_kernel_spmd(nc, in_maps, list(range(N_CORES)), trace=trace)
    out = np.zeros((S, D), dtype=np.float32)
    for c in range(N_CORES):
        out += res.results[c]["out"]
    return out.reshape(1, S, D), res


def kernel(x, wq, wk, wv, wo, freqs_cos, freqs_sin, mask, start_pos):
    out, _ = _run(x, wq, wk, wv, wo, freqs_cos, freqs_sin, mask, start_pos)
    return out
